# revision 70
# baseline (speedup 1.0000x reference)
"""Trainium2 Bass kernel for a dense transformer block (B=2, T=2048, C=1024,
H=16 heads, HS=64, FF=4096, fp32), SPMD across 8 NeuronCores.

Sharding strategy (v3)
----------------------
- LayerNorms + FFN + proj: sequence-parallel; core c owns 512 tokens.
- Attention: head-parallel; core c owns heads 2c, 2c+1 over all tokens.
- QKV is computed token-sharded (each core projects its OWN 512 tokens
  through ALL 16 heads' Q/K/V); results are re-sharded head-wise with
  three pipelined AllToAlls (v first - it absorbs the cross-core launch
  skew - then qk-even, then qk-odd which lands during head-pair-0
  attention).  Staging DMAs are pipelined per-destination behind the
  QKV matmuls; four qk-odd compute units are dep-gated on the first two
  AllToAlls so the PE array stays busy through the collective window
  and enters attention with the HAM clock-gate warm.
- Attention scores: the two key-tiles of each [128,1024] PSUM pair run
  as CONCURRENT row-tiled matmuls on the upper/lower 64-row strips of
  the PE array (q/k duplicated to partitions 64-127), halving QK time
  and keeping attention ACT(exp)-bound even when the HAM clock-gate
  holds the PE at 1.2GHz.  Each diagonal tile is paired as slot 0 with
  a fully-computed partner so exp/QK/mask/PV skip the leading fully
  masked query columns as one contiguous range.
- att^T returns to token-sharding with one AllToAll per local head; the
  even-head half of the output projection overlaps the second AllToAll.

Numerics: matmul operands bf16 (fp32 PSUM accumulate); LayerNorm stats,
softmax normalization, residuals in fp32. LN scale/bias and the per-head
attention scale fold into the weights on the host; K-bias dropped
(softmax is invariant to per-query constant offsets).
"""

import os
import numpy as np

B, T, C = 2, 2048, 1024
H, HS = 16, 64
FF = 4 * C
EPS = 1e-5
NCORE = 8
TOK = B * T            # 4096 flattened tokens
CHUNK = TOK // NCORE   # 512 tokens per core
P = 128
NTT = CHUNK // P       # 4 token tiles of 128 per core
NG = C // P            # 8 channel chunks
NF = FF // P           # 32 ff slices
LH = 2                 # local heads per core

_BUILT = None


def _build():
    import concourse.bass as bass
    import concourse.tile as tile
    from bass_rust import add_dep_helper
    from concourse import bacc, mybir
    from concourse.masks import make_identity
    from contextlib import ExitStack

    f32 = mybir.dt.float32
    bf16 = mybir.dt.bfloat16
    Alu = mybir.AluOpType
    Act = mybir.ActivationFunctionType

    nc = bacc.Bacc("TRN2", target_bir_lowering=False, debug=False,
                   num_devices=NCORE)

    xc = nc.dram_tensor("xc", [P, NTT, C], f32, kind="ExternalInput").ap()
    # wqkv blocks: [0]=qk head 2d, [1]=v (heads 2d,2d+1 per dst d), [2]=qk 2d+1
    # packed dst-major so per-dst chunks load contiguously
    wqkv = nc.dram_tensor("wqkv", [3, P, NCORE, NG, P], bf16,
                          kind="ExternalInput").ap()
    bqk0 = nc.dram_tensor("bqk0", [P, NCORE], f32, kind="ExternalInput").ap()
    bqk1 = nc.dram_tensor("bqk1", [P, NCORE], f32, kind="ExternalInput").ap()
    bv = nc.dram_tensor("bv", [P, NCORE], f32, kind="ExternalInput").ap()
    wproj = nc.dram_tensor("wproj", [P, NG, C], bf16,
                           kind="ExternalInput").ap()
    w1 = nc.dram_tensor("w1", [P, NG, FF], bf16, kind="ExternalInput").ap()
    bff1 = nc.dram_tensor("bff1", [P, NF], f32, kind="ExternalInput").ap()
    w2 = nc.dram_tensor("w2", [2, 4, P, 8, 512], bf16,
                        kind="ExternalInput").ap()
    out = nc.dram_tensor("out", [CHUNK, C], f32, kind="ExternalOutput").ap()
    DEBUG = bool(int(os.environ.get("BASSK_DEBUG", "0")))
    if DEBUG:
        dbg_qkT = nc.dram_tensor("dbg_qkT", [2, 2, 64, TOK], bf16,
                                 kind="ExternalOutput").ap()
        dbg_v = nc.dram_tensor("dbg_v", [P, TOK // P, 132], bf16,
                               kind="ExternalOutput").ap()
        dbg_xmid = nc.dram_tensor("dbg_xmid", [P, NTT, C], f32,
                                  kind="ExternalOutput").ap()

    # three pipelined AllToAlls: v first (absorbs cross-core launch skew),
    # then qk-even, then qk-odd (lands during head-pair-0 attention)
    v_b = nc.dram_tensor("v_b", [NCORE, P, NTT, 132], bf16)
    v_r = nc.dram_tensor("v_r", [NCORE, P, NTT, 132], bf16)
    qk0_b = nc.dram_tensor("qk0_b", [NCORE, P, CHUNK], bf16)
    qk0_r = nc.dram_tensor("qk0_r", [NCORE, P, CHUNK], bf16)
    qk1_b = nc.dram_tensor("qk1_b", [NCORE, P, CHUNK], bf16)
    qk1_r = nc.dram_tensor("qk1_r", [NCORE, P, CHUNK], bf16)
    attT_bounce = [nc.dram_tensor(f"attT_bounce{i}", [NCORE, 64, CHUNK], bf16)
                   for i in range(LH)]
    attT_recv = [nc.dram_tensor(f"attT_recv{i}", [NCORE, 64, CHUNK], bf16)
                 for i in range(LH)]
    groups = [list(range(NCORE))]

    with tile.TileContext(nc) as tc, ExitStack() as top:
        const = top.enter_context(tc.tile_pool(name="const", bufs=1))
        persist = top.enter_context(tc.tile_pool(name="persist", bufs=1))
        ps = top.enter_context(tc.tile_pool(name="ps", bufs=4, space="PSUM"))
        ps2 = top.enter_context(tc.tile_pool(name="ps2", bufs=2, space="PSUM"))

        ident = const.tile([P, P], bf16)
        make_identity(nc, ident)
        eps_sb = const.tile([P, 1], f32)
        nc.vector.memset(eps_sb, EPS)
        # causal mask tiles: mask[m][p, col] = col >= p + 128*m
        masks = const.tile([P, 4, 512], bf16)
        nc.vector.memset(masks, 1.0)
        for m in range(4):
            nc.gpsimd.affine_select(
                out=masks[:, m, :], in_=masks[:, m, :], pattern=[[1, 512]],
                compare_op=Alu.is_ge, fill=0.0, base=-128 * m,
                channel_multiplier=-1)

        xc_sb = persist.tile([P, NTT, C], f32)
        xmid_sb = persist.tile([P, NTT, C], f32)
        bqk0_sb = persist.tile([P, NCORE], f32)
        bqk1_sb = persist.tile([P, NCORE], f32)
        bv_sb = persist.tile([P, NCORE], f32)
        bff1_sb = persist.tile([P, NF], f32)
        w1p = top.enter_context(tc.tile_pool(name="w1p", bufs=1))
        w1_sb = w1p.tile([P, NG, FF], bf16)   # prefetched during attention
        prp = top.enter_context(tc.tile_pool(name="prp", bufs=1))
        wpp = top.enter_context(tc.tile_pool(name="wpp", bufs=1))
        ats0 = prp.tile([P, 4, CHUNK], bf16, name="ats0")
        ats1 = prp.tile([P, 4, CHUNK], bf16, name="ats1")
        wp = wpp.tile([P, NG, C], bf16)

        # startup: xc tiles on the sync queue, weight blocks concurrently on
        # the scalar queue, so LN1 and the first QKV matmuls start early
        for jt in range(NTT):
            nc.sync.dma_start(out=xc_sb[:, jt, :], in_=xc[:, jt, :])
        nc.sync.dma_start(out=bqk0_sb, in_=bqk0)
        nc.sync.dma_start(out=bqk1_sb, in_=bqk1)
        nc.sync.dma_start(out=bv_sb, in_=bv)
        nc.sync.dma_start(out=bff1_sb, in_=bff1)

        def layernorm_tile(pool, src_ap, out_dt):
            """src_ap: [P, C] fp32 in SBUF -> normalized [P, C] tile."""
            stats = pool.tile([P, 2, 6], f32, tag="ln_stats")
            nc.vector.bn_stats(out=stats[:, 0, :], in_=src_ap[:, 0:512])
            nc.vector.bn_stats(out=stats[:, 1, :], in_=src_ap[:, 512:1024])
            mv = pool.tile([P, 2], f32, tag="ln_mv")
            nc.vector.bn_aggr(out=mv, in_=stats)
            rstd = pool.tile([P, 1], f32, tag="ln_rstd")
            nc.scalar.activation(rstd, mv[:, 1:2], Act.Sqrt, bias=eps_sb)
            nc.vector.reciprocal(rstd, rstd)
            negmr = pool.tile([P, 1], f32, tag="ln_negmr")
            nc.vector.tensor_scalar(negmr, mv[:, 0:1], rstd, -1.0,
                                    Alu.mult, Alu.mult)
            hn = pool.tile([P, C], out_dt, tag="ln_out")
            nc.scalar.activation(hn, src_ap, Act.Identity,
                                 bias=negmr, scale=rstd)
            return hn

        # ---------------- Stage A: LN1 + transpose (own chunk) --------------
        with ExitStack() as sa:
            lnp = sa.enter_context(tc.tile_pool(name="lnp", bufs=3))
            qkvp = sa.enter_context(tc.tile_pool(name="qkvp", bufs=1))
            wqp = sa.enter_context(tc.tile_pool(name="wqp", bufs=2))

            hT_sb = qkvp.tile([P, NG, CHUNK], bf16)
            wv_sb = wqp.tile([P, NCORE, NG, P], bf16, tag="wblk",
                             name="wv_sb")
            for d in range(NCORE):
                nc.scalar.dma_start(out=wv_sb[:, d], in_=wqkv[1, :, d])
            wq0_sb = wqp.tile([P, NCORE, NG, P], bf16, tag="wblk",
                              name="wq0_sb")

            for jt in range(NTT):
                hn = layernorm_tile(lnp, xc_sb[:, jt, :], bf16)
                for g in range(NG):
                    tp = ps.tile([P, P], bf16, tag="bank")
                    nc.tensor.transpose(tp, hn[:, P * g:P * (g + 1)], ident)
                    nc.vector.tensor_copy(
                        hT_sb[:, g, P * jt:P * (jt + 1)], tp)

            # -------- Stage A2: QKV for all heads, one combined AllToAll ----
            stg = sa.enter_context(tc.tile_pool(name="stg", bufs=1))

            # phase 0: V (both heads of dst d), pre-transposed to [token, ch]
            # dst d+1's matmuls are emitted before dst d's transposes so the
            # PE never stalls on the bias-add; transposes borrow ps2 banks
            vT_st = stg.tile([P, NCORE, NTT, 132], bf16, name="vT_st")
            nc.vector.memset(vT_st.rearrange(
                "p d t (h u) -> p d t h u", h=2)[:, :, :, :, 64:65], 1.0)
            vts = [None] * NCORE
            bav = [None] * NCORE
            for d in range(NCORE + 1):
                if d < NCORE:
                    psV = ps.tile([P, CHUNK], f32, tag="bank")
                    for g in range(NG):
                        nc.tensor.matmul(psV, wv_sb[:, d, g, :],
                                         hT_sb[:, g, :], start=(g == 0),
                                         stop=(g == NG - 1))
                    vt = lnp.tile([P, CHUNK], bf16, tag="vt")
                    bav[d] = nc.vector.tensor_scalar_add(
                        vt, psV, bv_sb[:, d:d + 1])
                    vts[d] = vt
                if d > 0:
                    dp = d - 1
                    for tt in range(NTT):
                        tpv = ps2.tile([P, P], bf16, tag="bank2")
                        nc.tensor.transpose(
                            tpv, vts[dp][:, P * tt:P * (tt + 1)], ident)
                        nc.vector.tensor_copy(
                            vT_st[:, dp, tt, :].rearrange("p (h u) -> p h u",
                                                          h=2)[:, :, 0:64],
                            tpv.rearrange("p (h u) -> p h u", h=2))
                    nc.sync.dma_start(
                        out=v_b[dp], in_=vT_st[:, dp, :, :])
            cc_v = nc.gpsimd.collective_compute(
                "AllToAll", Alu.bypass, replica_groups=groups,
                ins=[v_b[:, :, :, :]], outs=[v_r[:, :, :, :]])

            # deferred weight loads: keep the HBM bus clear for xc/wv at
            # startup; each block lands just before its compute needs it
            for d in range(NCORE):
                wq0_dma = nc.scalar.dma_start(out=wq0_sb[:, d],
                                              in_=wqkv[0, :, d])
                add_dep_helper(wq0_dma.ins, bav[0].ins, sync=True,
                               reason="wq0 load behind V compute")

            # phase 1: q/k of even heads (head 2d -> dst core d)
            qk0_st = stg.tile([P, NCORE, CHUNK], bf16, name="qk0_st")
            for d in range(NCORE):
                psA = ps.tile([P, CHUNK], f32, tag="bank")
                for g in range(NG):
                    nc.tensor.matmul(psA, wq0_sb[:, d, g, :],
                                     hT_sb[:, g, :], start=(g == 0),
                                     stop=(g == NG - 1))
                nc.vector.tensor_scalar_add(qk0_st[:, d, :], psA,
                                            bqk0_sb[:, d:d + 1])
                nc.sync.dma_start(out=qk0_b[d, :, :], in_=qk0_st[:, d, :])
            cc_qk0 = nc.gpsimd.collective_compute(
                "AllToAll", Alu.bypass, replica_groups=groups,
                ins=[qk0_b[:, :, :]], outs=[qk0_r[:, :, :]])

            wq1_sb = wqp.tile([P, NCORE, NG, P], bf16, tag="wblk",
                              name="wq1_sb")
            for d in range(NCORE):
                wq1_dma = nc.scalar.dma_start(out=wq1_sb[:, d],
                                              in_=wqkv[2, :, d])
                add_dep_helper(wq1_dma.ins, bav[4].ins, sync=True,
                               reason="wq1 load behind V compute")

            # phase 2: q/k of odd heads — dst 0-3 computed while the V
            # AllToAll flies; dst 4-5 and 6-7 are gated on the V and qk0
            # AllToAlls so the PE stays busy through the collective window
            # and enters attention with the HAM clock-gate warm (a cold
            # entry runs a whole head-pair at 1.2GHz instead of 2.4GHz).
            qk1_st = stg.tile([P, NCORE, CHUNK], bf16, name="qk1_st")
            for d in range(NCORE):
                psA = ps.tile([P, CHUNK], f32, tag="bank")
                for g in range(NG):
                    mm = nc.tensor.matmul(psA, wq1_sb[:, d, g, :],
                                          hT_sb[:, g, :], start=(g == 0),
                                          stop=(g == NG - 1))
                    if d == 4 and g == 0:
                        add_dep_helper(mm.ins, cc_v.ins, sync=True,
                                       reason="warm-entry bridge work 1")
                    if d == 5 and g == 0:
                        # three units (~10us) after the qk0 AllToAll: long
                        # enough to span its completion-semaphore lag plus
                        # the unpack loads, so attention enters warm
                        add_dep_helper(mm.ins, cc_qk0.ins, sync=True,
                                       reason="warm-entry bridge work 2")
                nc.vector.tensor_scalar_add(qk1_st[:, d, :], psA,
                                            bqk1_sb[:, d:d + 1])
                # d6/d7 staging follows the bridge-gated compute; it rides
                # the gpsimd queue so it cannot delay the batch-0 q/k
                # unpack transfers on the scalar/sync queues
                eng = nc.gpsimd if d >= 6 else nc.sync
                eng.dma_start(out=qk1_b[d, :, :], in_=qk1_st[:, d, :])
            cc_qk1 = nc.gpsimd.collective_compute(
                "AllToAll", Alu.bypass, replica_groups=groups,
                ins=[qk1_b[:, :, :]], outs=[qk1_r[:, :, :]])

        # ---------------- Stage B: attention --------------------------------
        with ExitStack() as sb:
            qkp = sb.enter_context(tc.tile_pool(name="qkp", bufs=1))

            # q/k live on partitions 0-63 and are duplicated onto 64-127 so
            # the two key-tiles of each score pair run as concurrent
            # row-tiled matmuls on the upper/lower halves of the PE array
            # (halves QK time; keeps attention ACT-bound even at 1.2GHz).
            # Tiles are per-(head,batch): tile-granularity dependency
            # tracking then lets batch-0 attention start before batch-1's
            # unpack loads have landed.
            qT = [[qkp.tile([P, T], bf16, name=f"qT{i}_{b}") for b in (0, 1)]
                  for i in range(LH)]
            kT = [[qkp.tile([P, T], bf16, name=f"kT{i}_{b}") for b in (0, 1)]
                  for i in range(LH)]
            Vsb = qkp.tile([P, TOK // P, 132], bf16)

            # spread the unpack loads over three queues; each load rides
            # out during the NEXT AllToAll so attention starts immediately.
            # qT0/kT0 are split in half across queues — these strided
            # unpacks are DMA-descriptor-rate-bound, not bandwidth-bound.
            # batch-0's q/k (src cores 0-3) ride the scalar/gpsimd queues;
            # batch-1 and all head-pair-1 loads ride sync so NOTHING sits
            # on the scalar queue between here and the attention exps (a
            # load waiting on a later AllToAll would block them in FIFO
            # order).
            nc.gpsimd.dma_start(
                out=Vsb.rearrange("p (d t) c -> p d t c", d=NCORE),
                in_=v_r[:, :, :, :].rearrange("d p t c -> p d t c"))

            def qk_loads(eng, qt, kt, rcv, dlo, dhi, keng=None):
                cols = slice(512 * dlo % T, 512 * dlo % T + 512 * (dhi - dlo))
                dmas = []
                for tdst, plo, e in ((qt, 0, eng), (kt, 64, keng or eng)):
                    e.dma_start(
                        out=tdst[0:64, cols].rearrange(
                            "p (d m) -> p d m", d=dhi - dlo),
                        in_=rcv[dlo:dhi, plo:plo + 64, :].rearrange(
                            "d p m -> p d m"))
                    dmas.append(e.dma_start(out=tdst[64:128, cols],
                                            in_=tdst[0:64, cols]))
                return dmas

            # batch-0 of head-pair 0 is the attention-start critical path:
            # q rides scalar while k rides sync, transferring in parallel
            qk_loads(nc.scalar, qT[0][0], kT[0][0], qk0_r, 0, 4,
                     keng=nc.sync)
            qk_loads(nc.sync, qT[0][1], kT[0][1], qk0_r, 4, 8)
            _, kt1_dma = qk_loads(nc.sync, qT[1][0], kT[1][0], qk1_r, 0, 4)
            qt1_dma = qk_loads(nc.sync, qT[1][1], kT[1][1], qk1_r, 4, 8)[1]
            # w1 prefetch (8MB): on the sync queue behind the last unpack
            # load, keeping the gpsimd DMA queue free for attT staging
            w1_dma = nc.sync.dma_start(out=w1_sb, in_=w1)
            add_dep_helper(w1_dma.ins, qt1_dma.ins, sync=True,
                           reason="w1 load during attention")
            if DEBUG:
                for hp in range(LH):
                    for b in range(B):
                        nc.scalar.dma_start(out=dbg_qkT[hp, 0, :, T * b:
                                                        T * (b + 1)],
                                            in_=qT[hp][b][0:64])
                        nc.scalar.dma_start(out=dbg_qkT[hp, 1, :, T * b:
                                                        T * (b + 1)],
                                            in_=kT[hp][b][0:64])
                nc.scalar.dma_start(out=dbg_v, in_=Vsb)

            # attention: per local head hp, batch b, query tile jq (512 wide).
            # Key tiles are processed in pairs sharing one [P,1024] PSUM set
            # and one exp; each diagonal tile is paired as slot 0 with a
            # fully-computed partner as slot 1, so the exp (and the slot-0
            # QK matmul, mask, and PV matmul) skip the leading fully-masked
            # query columns while staying one contiguous range.
            atp = sb.enter_context(tc.tile_pool(name="atp", bufs=4))
            ate = sb.enter_context(tc.tile_pool(name="ate", bufs=2))
            for hp in range(LH):
                for b in range(B):
                    base_t = T * b
                    for jq in range(4):
                        q0 = base_t + 512 * jq
                        if jq == 0:
                            pairs = [(3, 2, 384), (1, 0, 128)]
                        else:
                            pairs = [(4 * jq + 3, 0, 384),
                                     (4 * jq + 2, 1, 256),
                                     (4 * jq + 1, 2, 128),
                                     (4 * jq + 0, 3, 0)]
                            offs = list(range(4, 4 * jq))
                            pairs += [(offs[i], offs[i + 1], 0)
                                      for i in range(0, len(offs), 2)]
                        psPV = ps.tile([65, 512], f32, tag="bank")
                        q0l = 512 * jq
                        for pi, (ika, ikb, trim) in enumerate(pairs):
                            psS2 = ps2.tile([P, 1024], f32, tag="bank2")
                            for dd, ik in ((0, ika), (1, ikb)):
                                k0 = P * ik
                                lo = trim if dd == 0 else 0
                                rb = 64 * dd
                                nc.tensor.matmul(
                                    psS2[:, 512 * dd + lo:512 * (dd + 1)],
                                    kT[hp][b][rb:rb + 64, k0:k0 + P],
                                    qT[hp][b][rb:rb + 64, q0l + lo:q0l + 512],
                                    start=True, stop=True,
                                    tile_position=(rb, 0))
                            pt = atp.tile([P, 1024], bf16, tag="pt")
                            nc.scalar.activation(pt[:, trim:], psS2[:, trim:],
                                                 Act.Exp)
                            for dd, ik in ((0, ika), (1, ikb)):
                                m = ik - 4 * jq
                                lo = 128 * m if m >= 0 else 0
                                ph = pt[:, 512 * dd + lo:512 * (dd + 1)]
                                if m >= 0:  # diagonal block: causal mask
                                    nc.vector.tensor_mul(
                                        ph, ph, masks[:, m, lo:])
                                nc.tensor.matmul(
                                    psPV[:, lo:], Vsb[:, (base_t // P) + ik,
                                                      66 * hp:66 * hp + 65],
                                    ph, start=(pi == 0 and dd == 0),
                                    stop=(pi == len(pairs) - 1 and dd == 1))
                        rs = ate.tile([1, 512], f32, tag="rs")
                        nc.vector.tensor_copy(rs, psPV[64:65, :])
                        rec_f = ate.tile([1, 512], f32, tag="rec_f")
                        nc.vector.reciprocal_approx_fast(rec_f, rs)
                        bc = ate.tile([64, 512], f32, tag="bc")
                        nc.gpsimd.partition_broadcast(bc, rec_f[0:1, :])
                        att = ate.tile([64, 512], bf16, tag="attout")
                        nc.vector.tensor_mul(att, psPV[0:64, :], bc)
                        nc.gpsimd.dma_start(
                            out=attT_bounce[hp][4 * b + jq, :, :],
                            in_=att)
                if b == B - 1:
                    nc.gpsimd.collective_compute(
                        "AllToAll", Alu.bypass, replica_groups=groups,
                        ins=[attT_bounce[hp][:, :, :]],
                        outs=[attT_recv[hp][:, :, :]])
            tc.no_sync_barrier()

        # ---------------- Stage C: proj + residual --------------------------
        # Wproj rows host-permuted: first 512 = even-head channels, last 512
        # = odd. The even half only needs attT_recv[0], so it runs while the
        # second AllToAll flies.
        with ExitStack() as sc:
            wp_dma = nc.sync.dma_start(out=wp, in_=wproj)
            add_dep_helper(wp_dma.ins, kt1_dma.ins, sync=True,
                           reason="wproj load during attention")
            # ats1 is on the post-attention critical path: split it across
            # two queues (scalar is idle once the exps are done)
            for hp, dstt, eng2 in ((0, ats0, nc.sync), (1, ats1, nc.scalar)):
                rv = attT_recv[hp][:, :, :].rearrange(
                    "(gg two) p m -> p gg two m", two=2)
                nc.sync.dma_start(out=dstt[0:64, :, :], in_=rv[:, :, 0, :])
                eng2.dma_start(out=dstt[64:128, :, :], in_=rv[:, :, 1, :])
            # 8 accumulators: 4 single-bank + 2 double-bank halves
            pA = [ps.tile([P, 512], f32, tag="bank", name=f"prA{i}")
                  for i in range(4)]
            pB = [ps2.tile([P, 1024], f32, tag="bank2", name=f"prB{i}")
                  for i in range(2)]
            acc = pA + [pB[0][:, 0:512], pB[0][:, 512:1024],
                        pB[1][:, 0:512], pB[1][:, 512:1024]]
            for n in range(2):
                for jt in range(NTT):
                    for gg in range(4):
                        nc.tensor.matmul(
                            acc[4 * n + jt],
                            ats0[:, gg, P * jt:P * (jt + 1)],
                            wp[:, gg, 512 * n:512 * (n + 1)],
                            start=(gg == 0), stop=False)
            # odd half jt-major with the residual add inline, so xmid token
            # tiles complete one by one and LN2 overlaps the rest of proj
            for jt in range(NTT):
                for n in range(2):
                    for gg in range(4):
                        nc.tensor.matmul(
                            acc[4 * n + jt],
                            ats1[:, gg, P * jt:P * (jt + 1)],
                            wp[:, 4 + gg, 512 * n:512 * (n + 1)],
                            start=False, stop=(gg == 3))
                for n in range(2):
                    nc.vector.tensor_add(
                        xmid_sb[:, jt, 512 * n:512 * (n + 1)], acc[4 * n + jt],
                        xc_sb[:, jt, 512 * n:512 * (n + 1)])

        if DEBUG:
            nc.sync.dma_start(out=dbg_xmid, in_=xmid_sb)

        # ---------------- Stage D: LN2 + FFN + residual ---------------------
        with ExitStack() as sd:
            ffp = sd.enter_context(tc.tile_pool(name="ffp", bufs=1))
            lnp2 = sd.enter_context(tc.tile_pool(name="lnp2", bufs=3))
            w2p = sd.enter_context(tc.tile_pool(name="w2p", bufs=2))
            outp = sd.enter_context(tc.tile_pool(name="outp", bufs=3))

            h2T = ffp.tile([P, NG, CHUNK], bf16)
            ff1T = ffp.tile([P, NF, CHUNK], bf16)

            for jt in range(NTT):
                hn2 = layernorm_tile(lnp2, xmid_sb[:, jt, :], bf16)
                for g in range(NG):
                    tp = ps.tile([P, P], bf16, tag="bank")
                    nc.tensor.transpose(tp, hn2[:, P * g:P * (g + 1)], ident)
                    nc.vector.tensor_copy(
                        h2T[:, g, P * jt:P * (jt + 1)], tp)

            for f in range(NF):
                psF = ps2.tile([P, CHUNK], f32, tag="bank2")
                for g in range(NG):
                    nc.tensor.matmul(psF, w1_sb[:, g, P * f:P * (f + 1)],
                                     h2T[:, g, :],
                                     start=(g == 0), stop=(g == NG - 1))
                nc.scalar.activation(ff1T[:, f, :], psF, Act.Relu,
                                     bias=bff1_sb[:, f:f + 1])

            # FFN2: w2 loaded in 1MB quarters, double-buffered
            for n in range(2):
                psj = [ps.tile([P, 512], f32, tag="bank", name=f"psk{n}_{jt}")
                       for jt in range(NTT)]
                for a in range(4):
                    w2q = w2p.tile([P, 8, 512], bf16, tag="w2q")
                    nc.sync.dma_start(out=w2q, in_=w2[n, a])
                    if a < 3:
                        for ql in range(8):
                            q = 8 * a + ql
                            for jt in range(NTT):
                                nc.tensor.matmul(
                                    psj[jt], ff1T[:, q, P * jt:P * (jt + 1)],
                                    w2q[:, ql, :], start=(q == 0), stop=False)
                    else:
                        # jt-major so early tiles finish; add+store overlap
                        for jt in range(NTT):
                            for ql in range(8):
                                nc.tensor.matmul(
                                    psj[jt],
                                    ff1T[:, 8 * a + ql,
                                         P * jt:P * (jt + 1)],
                                    w2q[:, ql, :], start=False,
                                    stop=(ql == 7))
                            ot = outp.tile([P, 512], f32, tag="outt")
                            nc.vector.tensor_add(
                                ot, psj[jt],
                                xmid_sb[:, jt, 512 * n:512 * (n + 1)])
                            # scalar queue is idle during FFN2 and its
                            # hardware DMA path beats gpsimd's SWDGE
                            nc.scalar.dma_start(
                                out=out[P * jt:P * (jt + 1),
                                        512 * n:512 * (n + 1)],
                                in_=ot)

    nc.compile()
    return nc


def _pack_pg(w):
    """[C, M] -> [P, C//P, M] partition-major packing."""
    Cr, M = w.shape
    return np.ascontiguousarray(
        w.reshape(Cr // P, P, M).transpose(1, 0, 2))


def _prepare_inputs(x, Wq, Wk, Wv, p, Wproj, W1, W2,
                    ln1_w, ln1_b, ln2_w, ln2_b):
    import ml_dtypes
    f = np.float32
    bf = ml_dtypes.bfloat16
    x = np.asarray(x, f).reshape(TOK, C)
    Wq, Wk, Wv = (np.asarray(a, f) for a in (Wq, Wk, Wv))
    p = np.asarray(p, f)
    Wproj = np.asarray(Wproj, f)
    W1, W2 = np.asarray(W1, f), np.asarray(W2, f)
    ln1_w, ln1_b = np.asarray(ln1_w, f), np.asarray(ln1_b, f)
    ln2_w, ln2_b = np.asarray(ln2_w, f), np.asarray(ln2_b, f)

    s = (p.astype(np.float64) ** -0.5).astype(f)

    w1_p = _pack_pg((ln2_w[:, None] * W1).astype(bf))
    bff1 = ln2_b @ W1
    bff1 = np.ascontiguousarray(bff1.reshape(NF, P).T.astype(f))
    # w2 quarters: [2 n, 4 a, P, 8, 512]
    w2_bf = W2.astype(bf)
    w2_p = np.empty((2, 4, P, 8, 512), bf)
    for n in range(2):
        for a in range(4):
            blk = w2_bf[1024 * a:1024 * (a + 1), 512 * n:512 * (n + 1)]
            w2_p[n, a] = blk.reshape(8, P, 512).transpose(1, 0, 2)
    # Wproj rows permuted: even-head channels first, then odd
    ev = np.arange(C).reshape(H, HS)[0::2].ravel()
    od = np.arange(C).reshape(H, HS)[1::2].ravel()
    wproj_p = _pack_pg(
        np.concatenate([Wproj[ev], Wproj[od]], axis=0).astype(bf))

    # wqkv blocks: [qk-even | v | qk-odd], packed per block
    vblk = np.concatenate(
        [np.concatenate([ln1_w[:, None] * Wv[2 * d],
                         ln1_w[:, None] * Wv[2 * d + 1]], axis=1)
         for d in range(NCORE)], axis=1)
    qk0blk = np.concatenate(
        [np.concatenate([ln1_w[:, None] * Wq[2 * d] * s[2 * d],
                         ln1_w[:, None] * Wk[2 * d]], axis=1)
         for d in range(NCORE)], axis=1)
    qk1blk = np.concatenate(
        [np.concatenate([ln1_w[:, None] * Wq[2 * d + 1] * s[2 * d + 1],
                         ln1_w[:, None] * Wk[2 * d + 1]], axis=1)
         for d in range(NCORE)], axis=1)
    def _pack_dmaj(w):
        """[C, C] -> [P, NCORE, NG, P] dst-major packing."""
        return np.ascontiguousarray(
            w.reshape(NG, P, NCORE, P).transpose(1, 2, 0, 3))

    wqkv_p = np.stack([_pack_dmaj(qk0blk.astype(bf)),
                       _pack_dmaj(vblk.astype(bf)),
                       _pack_dmaj(qk1blk.astype(bf))])

    # K bias intentionally zero: softmax is invariant to it
    bqk0 = np.stack([np.concatenate([s[2 * d] * (ln1_b @ Wq[2 * d]),
                                     np.zeros(HS, f)]) for d in range(NCORE)],
                    axis=1)
    bqk1 = np.stack([np.concatenate([s[2 * d + 1] * (ln1_b @ Wq[2 * d + 1]),
                                     np.zeros(HS, f)]) for d in range(NCORE)],
                    axis=1)
    bv_a = np.stack([np.concatenate([ln1_b @ Wv[2 * d],
                                     ln1_b @ Wv[2 * d + 1]])
                     for d in range(NCORE)], axis=1)

    shared = {
        "wqkv": np.ascontiguousarray(wqkv_p),
        "bqk0": np.ascontiguousarray(bqk0.astype(f)),
        "bqk1": np.ascontiguousarray(bqk1.astype(f)),
        "bv": np.ascontiguousarray(bv_a.astype(f)),
        "wproj": wproj_p,
        "w1": w1_p,
        "bff1": bff1,
        "w2": np.ascontiguousarray(w2_p),
    }
    in_maps = []
    for c in range(NCORE):
        m = dict(shared)
        xch = x[CHUNK * c:CHUNK * (c + 1)]
        m["xc"] = np.ascontiguousarray(
            xch.reshape(NTT, P, C).transpose(1, 0, 2))
        in_maps.append(m)
    return in_maps


def kernel(**inputs):
    global _BUILT
    from concourse.bass_utils import run_bass_kernel_spmd

    if _BUILT is None:
        _BUILT = _build()
    in_maps = _prepare_inputs(**inputs)
    trace = bool(int(os.environ.get("BASSK_TRACE", "0")))
    res = run_bass_kernel_spmd(_BUILT, in_maps, list(range(NCORE)),
                               trace=trace)
    if trace:
        kernel.last_exec_time_ns = res.exec_time_ns
        kernel.last_res = res
    out = np.concatenate([res.results[c]["out"] for c in range(NCORE)], axis=0)
    return out.reshape(B, T, C).astype(np.float32)
